# revision 7
# baseline (speedup 1.0000x reference)
"""Trainium2 Bass kernel for nn_DQA_89077621719347 (dense_cnn, 8 cores).

Math (per batch b, channel c):
  feat_ave = mean_{h,w} feat                      # (b, c)
  CMA(feat_ave, deg) -> cma; emb = gamma*cma + deg
  kern = (lrelu(emb @ k_w1.T) @ k_w2.T)           # per-(b,c) 3x3 kernel
  z    = lrelu(depthwise3x3(feat, kern))
  out  = conv_w @ z + conv_b + feat * sigmoid(lrelu(deg@ca_w1.T)@ca_w2.T)

Sharding: data-parallel over batch, 2 batches/core -> 128 partitions=(b,c).
Depthwise conv = 9 diagonal-weight bf16 matmuls over shifted views of a
zero-padded bf16 copy of feat held in SBUF (width W+1: one shared pad col;
one zero row above/below).
"""
import numpy as np

import concourse.bass as bass
import concourse.bacc as bacc
import concourse.tile as tile
import concourse.mybir as mybir
from concourse.masks import make_identity

f32 = mybir.dt.float32
bf16 = mybir.dt.bfloat16
AF = mybir.ActivationFunctionType
OP = mybir.AluOpType

B, C, H, W = 16, 64, 256, 256
NCORES = 8
BPC = B // NCORES          # batches per core
P = BPC * C                # 128 partitions


def build_nc(h=H, w=W):
    """Build the per-core SPMD Bass module (shapes [BPC,C,h,w])."""
    pw = w + 1                 # padded row width (col 0 is the shared zero pad)
    ph = h + 2                 # zero row above and below
    cn = ph * pw + 2           # +2 tail margin for the (+1,+1) tap of last row
    npx = h * w
    n_groups = h // 2          # 2 output rows per group -> N=512 matmuls
    lr = min(16, h)            # image rows per pass-1 load slab
    n_slabs = h // lr

    nc = bacc.Bacc(trn_type="TRN2")

    feat = nc.dram_tensor("feat", [BPC, C, h, w], f32, kind="ExternalInput")
    deg = nc.dram_tensor("deg", [BPC, C], f32, kind="ExternalInput")
    wq = nc.dram_tensor("wq", [C, C], f32, kind="ExternalInput")
    bq = nc.dram_tensor("bq", [C], f32, kind="ExternalInput")
    wk = nc.dram_tensor("wk", [C, C], f32, kind="ExternalInput")
    bk = nc.dram_tensor("bk", [C], f32, kind="ExternalInput")
    wv = nc.dram_tensor("wv", [C, C], f32, kind="ExternalInput")
    bv = nc.dram_tensor("bv", [C], f32, kind="ExternalInput")
    gamma = nc.dram_tensor("gamma", [1], f32, kind="ExternalInput")
    k_w1 = nc.dram_tensor("k_w1", [C, C], f32, kind="ExternalInput")
    k_w2 = nc.dram_tensor("k_w2", [C * 9, C], f32, kind="ExternalInput")
    conv_w = nc.dram_tensor("conv_w", [C, C], f32, kind="ExternalInput")
    conv_b = nc.dram_tensor("conv_b", [C], f32, kind="ExternalInput")
    ca_w1 = nc.dram_tensor("ca_w1", [C // 8, C], f32, kind="ExternalInput")
    ca_w2 = nc.dram_tensor("ca_w2", [C, C // 8], f32, kind="ExternalInput")
    out = nc.dram_tensor("out", [BPC, C, h, w], f32, kind="ExternalOutput")

    featv = feat[:, :, :, :].rearrange("b c h w -> (b c) (h w)")
    outv = out[:, :, :, :].rearrange("b c h w -> (b c) (h w)")

    with tile.TileContext(nc) as tc:
        import contextlib
        ctx = contextlib.ExitStack()
        with ctx:
            sing = ctx.enter_context(tc.tile_pool(name="sing", bufs=1))
            work = ctx.enter_context(tc.tile_pool(name="work", bufs=3))
            dr = ctx.enter_context(tc.tile_pool(name="dr", bufs=1, space="DRAM"))
            ps_v = ctx.enter_context(tc.tile_pool(name="ps_v", bufs=1, space="PSUM"))
            ps_d = ctx.enter_context(tc.tile_pool(name="ps_d", bufs=3, space="PSUM"))
            ps_o = ctx.enter_context(tc.tile_pool(name="ps_o", bufs=2, space="PSUM"))

            # ---------------- constants / weight prep (no feat dependency) ---
            ident_b = sing.tile([128, 128], bf16)
            make_identity(nc, ident_b[:, :])

            def load_T(src_dram, rows, cols, name):
                """Transposed load: DRAM [rows, cols] -> SBUF [cols, rows]
                via a strided AP (tiny tensors; cost irrelevant)."""
                t = sing.tile([cols, rows], f32, tag=f"T{name}")
                ap = bass.AP(tensor=src_dram[:, :].tensor, offset=0,
                             ap=[[1, cols], [cols, rows]])
                nc.sync.dma_start(out=t[:, :], in_=ap)
                return t

            def blkdiag(tsb, rows, cols, dtype=f32, name=""):
                """[128,128] block-diagonal from tsb ([rows, cols]): one block
                per batch at (b*64, b*64)."""
                blk = sing.tile([128, 128], dtype, tag=f"blk{name}")
                nc.gpsimd.memset(blk[:, :], 0.0)
                nc.vector.tensor_copy(blk[0:rows, 0:cols], tsb[:, :])
                nc.sync.dma_start(out=blk[64:64 + rows, 64:64 + cols],
                                  in_=tsb[:, :])
                return blk

            wqT = load_T(wq, 64, 64, "wq")
            wkT = load_T(wk, 64, 64, "wk")
            wvT = load_T(wv, 64, 64, "wv")
            k_w1T = load_T(k_w1, 64, 64, "kw1")
            conv_wT = load_T(conv_w, 64, 64, "cw")
            ca_w1T = load_T(ca_w1, 8, 64, "ca1")      # [64, 8]
            ca_w2T = load_T(ca_w2, 64, 8, "ca2")      # [8, 64]
            k_w2T = load_T(k_w2, 576, 64, "kw2")      # [64, 576]

            BQ = blkdiag(wqT, 64, 64, name="q")
            BK = blkdiag(wkT, 64, 64, name="k")
            BV = blkdiag(wvT, 64, 64, name="v")
            BW1 = blkdiag(k_w1T, 64, 64, name="w1")
            BA1 = blkdiag(ca_w1T, 64, 8, name="a1")
            BA2 = blkdiag(ca_w2T, 8, 64, name="a2")
            conv_wTb = sing.tile([64, 64], bf16)
            nc.vector.tensor_copy(conv_wTb[:, :], conv_wT[:, :])
            CB = blkdiag(conv_wTb, 64, 64, dtype=bf16, name="cw")

            # per-partition vectors
            def bcast_c(src, name):
                t = sing.tile([128, 1], f32, tag=f"pc{name}")
                ap = bass.AP(tensor=src[:].tensor, offset=0,
                             ap=[[0, BPC], [1, C]])
                nc.gpsimd.dma_start(out=t[:, 0:1], in_=ap)
                return t

            bq_pc = bcast_c(bq, "bq")
            bk_pc = bcast_c(bk, "bk")
            bv_pc = bcast_c(bv, "bv")
            conv_b_pc = bcast_c(conv_b, "cb")
            gamma_pc = sing.tile([128, 1], f32)
            nc.gpsimd.dma_start(
                out=gamma_pc[:, 0:1],
                in_=bass.AP(tensor=gamma[:].tensor, offset=0, ap=[[0, 128], [1, 1]]))
            deg_pc = sing.tile([128, 1], f32)
            nc.sync.dma_start(out=deg_pc[:, 0:1],
                              in_=deg[:, :].rearrange("b c -> (b c)")
                              .rearrange("(p one) -> p one", one=1))

            def vec_mm(blk_w, rhs_pc, name):
                """[128,1] = blkdiag.T @ rhs (tiny matmul), result in PSUM."""
                p = ps_v.tile([128, 1], f32, tag="vec")
                nc.tensor.matmul(p[:, 0:1], blk_w[:, :], rhs_pc[:, 0:1],
                                 start=True, stop=True)
                return p

            def lrelu_vec(psum_in, name):
                """lrelu on a [128,1] psum -> sbuf f32: 0.9*relu(x) + 0.1*x."""
                tr = work.tile([128, 1], f32, tag=f"lr{name}")
                nc.scalar.activation(out=tr[:, 0:1], in_=psum_in[:, 0:1],
                                     func=AF.Relu, bias=0.0, scale=0.9)
                o = sing.tile([128, 1], f32, tag=f"lro{name}")
                nc.vector.scalar_tensor_tensor(o[:, 0:1], psum_in[:, 0:1], 0.1,
                                               tr[:, 0:1], op0=OP.mult,
                                               op1=OP.add)
                return o

            # ---------------- pass 1: load feat -> bf16 cache + row sums ----
            cache = sing.tile([128, cn], bf16)
            # zero pads: top row, col 0 of every row, bottom row + tail
            nc.gpsimd.memset(cache[:, 0:pw], 0.0)
            nc.gpsimd.memset(
                cache[:, pw:ph * pw].rearrange("p (h w) -> p h w", w=pw)[:, :, 0:1],
                0.0)
            nc.gpsimd.memset(cache[:, (ph - 1) * pw:cn], 0.0)

            partials = sing.tile([128, n_slabs], f32)
            for i in range(n_slabs):
                base = (i * lr + 1) * pw + 1
                dst = cache[:, base - 1:base - 1 + lr * pw] \
                    .rearrange("p (h w) -> p h w", w=pw)[:, :, 1:1 + w]
                src = featv[:, i * lr * w:(i + 1) * lr * w] \
                    .rearrange("p (h w) -> p h w", w=w)
                nc.gpsimd.dma_start(out=dst, in_=src)  # casting DMA f32->bf16
                nc.vector.reduce_sum(partials[:, i:i + 1], dst,
                                     axis=mybir.AxisListType.XY)

            # Collapse the many pass-1/weight-prep writers into one sync point
            # (walrus caps per-instruction sync waits).
            tc.strict_bb_all_engine_barrier()

            feat_ave = sing.tile([128, 1], f32)
            nc.vector.reduce_sum(feat_ave[:, 0:1], partials[:, :],
                                 axis=mybir.AxisListType.X)
            nc.vector.tensor_scalar_mul(feat_ave[:, 0:1], feat_ave[:, 0:1],
                                        1.0 / npx)

            # --------- channel attention (depends only on deg) ----------------
            a0 = vec_mm(BA1, deg_pc, "a0")
            t_pr = lrelu_vec(a0, "a0")
            a1 = vec_mm(BA2, t_pr, "a1")
            att_pc = sing.tile([128, 1], f32)
            nc.scalar.activation(out=att_pc[:, 0:1], in_=a1[:, 0:1],
                                 func=AF.Sigmoid, bias=0.0, scale=1.0)

            # --------- kk / v (depend only on deg) ---------------------------
            kk0 = vec_mm(BK, deg_pc, "kk")
            kk_pi = sing.tile([128, 1], f32)
            nc.scalar.activation(out=kk_pi[:, 0:1], in_=kk0[:, 0:1],
                                 func=AF.Identity, bias=bk_pc[:, 0:1], scale=1.0)
            v0 = vec_mm(BV, deg_pc, "v")
            v_pi = sing.tile([128, 1], f32)
            nc.scalar.activation(out=v_pi[:, 0:1], in_=v0[:, 0:1],
                                 func=AF.Identity, bias=bv_pc[:, 0:1], scale=1.0)

            def rep64(src_pc, name):
                """[128,1] (p=(b,i)) -> [128,64] tile whose row (b,j) is
                src[b*64 : b*64+64] (replicated across j)."""
                d = dr.tile([128], f32, tag=f"dr{name}")
                nc.sync.dma_start(out=d[:], in_=src_pc[:, 0:1])
                rep = sing.tile([128, 64], f32, tag=f"rep{name}")
                ap = bass.AP(tensor=d[:].tensor, offset=d[:].offset,
                             ap=[[64, BPC], [0, C], [1, C]])
                nc.sync.dma_start(out=rep[:, :], in_=ap)
                return rep

            kk_rep = rep64(kk_pi, "kk")
            v_rep = rep64(v_pi, "v")

            # ---------------- CMA + kernel-predictor MLP ---------------------
            q0 = vec_mm(BQ, feat_ave, "q")
            q_pj = sing.tile([128, 1], f32)
            nc.scalar.activation(out=q_pj[:, 0:1], in_=q0[:, 0:1],
                                 func=AF.Identity, bias=bq_pc[:, 0:1], scale=1.0)

            energy = sing.tile([128, C], f32)
            nc.vector.tensor_scalar_mul(energy[:, :], kk_rep[:, :], q_pj[:, 0:1])
            mx = sing.tile([128, 1], f32)
            nc.vector.reduce_max(mx[:, 0:1], energy[:, :],
                                 axis=mybir.AxisListType.X)
            nmx = sing.tile([128, 1], f32)
            nc.vector.tensor_scalar_mul(nmx[:, 0:1], mx[:, 0:1], -1.0)
            ee = sing.tile([128, C], f32)
            nc.scalar.activation(out=ee[:, :], in_=energy[:, :], func=AF.Exp,
                                 bias=nmx[:, 0:1], scale=1.0)
            es = sing.tile([128, 1], f32)
            nc.vector.reduce_sum(es[:, 0:1], ee[:, :], axis=mybir.AxisListType.X)
            erc = sing.tile([128, 1], f32)
            nc.vector.reciprocal(erc[:, 0:1], es[:, 0:1])
            attn = sing.tile([128, C], f32)
            nc.vector.tensor_scalar_mul(attn[:, :], ee[:, :], erc[:, 0:1])
            prod = sing.tile([128, C], f32)
            nc.vector.tensor_mul(prod[:, :], attn[:, :], v_rep[:, :])
            cma = sing.tile([128, 1], f32)
            nc.vector.reduce_sum(cma[:, 0:1], prod[:, :],
                                 axis=mybir.AxisListType.X)
            emb = sing.tile([128, 1], f32)
            nc.vector.scalar_tensor_tensor(emb[:, 0:1], cma[:, 0:1],
                                           gamma_pc[:, 0:1], deg_pc[:, 0:1],
                                           op0=OP.mult, op1=OP.add)

            hid0 = vec_mm(BW1, emb, "hid")
            hid_pc = lrelu_vec(hid0, "hid")
            # reorder [128,1] (p=(b,j)) -> [64,2] (j, b)
            hd = dr.tile([128], f32, tag="drhid")
            nc.sync.dma_start(out=hd[:], in_=hid_pc[:, 0:1])
            hid_jb = sing.tile([64, 2], f32)
            nc.sync.dma_start(
                out=hid_jb[:, :],
                in_=bass.AP(tensor=hd[:].tensor, offset=hd[:].offset,
                            ap=[[1, C], [C, BPC]]))

            # kern[b, u] = sum_j hid[j,b] * k_w2T[j,u]   (u = c*9 + t)
            k_bu = sing.tile([2, 576], f32)
            for hh in range(2):
                kp = ps_v.tile([2, 288], f32, tag="k1")
                nc.tensor.matmul(kp[:, :], hid_jb[:, :],
                                 k_w2T[:, hh * 288:(hh + 1) * 288],
                                 start=True, stop=True)
                nc.scalar.copy(k_bu[:, hh * 288:(hh + 1) * 288], kp[:, :])
            kd = dr.tile([2 * 576], f32, tag="drkern")
            nc.sync.dma_start(out=kd[:], in_=k_bu[:, :])
            k_tap = sing.tile([128, 9], f32)
            nc.sync.dma_start(
                out=k_tap[:, :],
                in_=bass.AP(tensor=kd[:].tensor, offset=kd[:].offset,
                            ap=[[576, BPC], [9, C], [1, 9]]))

            diags = []
            for t in range(9):
                dg = sing.tile([128, 128], bf16, tag=f"diag{t}")
                nc.vector.tensor_scalar_mul(dg[:, :], ident_b[:, :],
                                            k_tap[:, t:t + 1])
                diags.append(dg)

            # ---------------- main loop: 2 output rows per group -------------
            cap = cache[:, :]

            def slab(r0, off):
                # [128, 2, w] strided view at flat offset r0*pw + 1 + off
                base = r0 * pw + 1 + off
                return bass.AP(tensor=cap.tensor, offset=cap.offset + base,
                               ap=[list(cap.ap[0]), [pw, 2], [1, w]])

            for g in range(n_groups):
                r0 = 2 * g + 1
                pd = ps_d.tile([128, 2 * w], f32, tag="pd")
                for idx in range(9):
                    ky, kx = idx // 3, idx % 3
                    off = (ky - 1) * pw + (kx - 1)
                    nc.tensor.matmul(pd[:, :], diags[idx][:, :], slab(r0, off),
                                     start=(idx == 0), stop=(idx == 8))
                t_relu = work.tile([128, 2 * w], f32, tag="t_relu")
                nc.scalar.activation(out=t_relu[:, :], in_=pd[:, :],
                                     func=AF.Relu, bias=0.0, scale=0.9)
                y = work.tile([128, 2 * w], bf16, tag="y")
                nc.vector.scalar_tensor_tensor(y[:, :], pd[:, :], 0.1,
                                               t_relu[:, :], op0=OP.mult,
                                               op1=OP.add)
                po = ps_o.tile([128, 2 * w], f32, tag="po")
                nc.tensor.matmul(po[:, :], CB[:, :], y[:, :],
                                 start=True, stop=True)
                t1 = work.tile([128, 2 * w], f32, tag="t1")
                nc.scalar.activation(out=t1[:, :], in_=slab(r0, 0),
                                     func=AF.Identity, bias=conv_b_pc[:, 0:1],
                                     scale=att_pc[:, 0:1])
                out_s = work.tile([128, 2 * w], f32, tag="out_s")
                nc.vector.tensor_add(out_s[:, :], t1[:, :], po[:, :])
                nc.sync.dma_start(out=outv[:, g * 2 * w:(g + 1) * 2 * w],
                                  in_=out_s[:, :])

    nc.finalize()
    return nc


_NC_CACHE = {}


def _get_nc(h, w):
    if (h, w) not in _NC_CACHE:
        _NC_CACHE[(h, w)] = build_nc(h, w)
    return _NC_CACHE[(h, w)]


def kernel(**inputs):
    from concourse.bass_utils import run_bass_kernel_spmd

    feat = np.ascontiguousarray(inputs["feat"], dtype=np.float32)
    deg = np.ascontiguousarray(inputs["deg"], dtype=np.float32)
    b, c, h, w = feat.shape
    nc = _get_nc(h, w)

    shared = {k: np.ascontiguousarray(np.asarray(v), dtype=np.float32)
              for k, v in inputs.items() if k not in ("feat", "deg")}
    in_maps = []
    for k in range(NCORES):
        m = dict(shared)
        m["feat"] = feat[k * BPC:(k + 1) * BPC]
        m["deg"] = deg[k * BPC:(k + 1) * BPC]
        in_maps.append(m)

    res = run_bass_kernel_spmd(nc, in_maps, core_ids=list(range(NCORES)))
    return np.concatenate([r["out"] for r in res.results], axis=0)


# revision 12
# speedup vs baseline: 199.0146x; 199.0146x over previous
"""Trainium2 Bass kernel for nn_DQA_89077621719347 (dense_cnn, 8 cores).

Math (per batch b, channel c):
  feat_ave = mean_{h,w} feat                      # (b, c)
  CMA(feat_ave, deg) -> cma; emb = gamma*cma + deg
  kern = (lrelu(emb @ k_w1.T) @ k_w2.T)           # per-(b,c) 3x3 kernel
  z    = lrelu(depthwise3x3(feat, kern))
  out  = conv_w @ z + conv_b + feat * sigmoid(lrelu(deg@ca_w1.T)@ca_w2.T)

Sharding: data-parallel over batch, 2 batches/core -> 128 partitions=(b,c).
Depthwise conv = 9 diagonal-weight bf16 matmuls over shifted views of a
zero-padded bf16 copy of feat held in SBUF (width W+1: one shared pad col;
one zero row above/below).
"""
import contextlib

import numpy as np

import concourse.bass as bass
import concourse.bacc as bacc
import concourse.tile as tile
import concourse.mybir as mybir
from concourse.masks import make_identity

f32 = mybir.dt.float32
bf16 = mybir.dt.bfloat16
AF = mybir.ActivationFunctionType
OP = mybir.AluOpType

B, C, H, W = 16, 64, 256, 256
NCORES = 8
BPC = B // NCORES          # batches per core
P = BPC * C                # 128 partitions


def build_nc(h=H, w=W, loop_reps=1):
    """Build the per-core SPMD Bass module (shapes [BPC,C,h,w]).

    loop_reps>1 wraps the computation in a hardware For_i loop — used only
    for timing (per-iteration time = wall-clock delta / extra reps)."""
    pw = w + 1                 # padded row width (col 0 is the shared zero pad)
    ph = h + 2                 # zero row above and below
    cn = ph * pw + 2           # +2 tail margin for the (+1,+1) tap of last row
    npx = h * w
    n_groups = h // 2          # 2 output rows per group -> N=512 matmuls
    lr = min(16, h)            # image rows per pass-1 load slab
    n_slabs = h // lr

    nc = bacc.Bacc(trn_type="TRN2")

    feat = nc.dram_tensor("feat", [BPC, C, h, w], f32, kind="ExternalInput")
    deg = nc.dram_tensor("deg", [BPC, C], f32, kind="ExternalInput")
    wq = nc.dram_tensor("wq", [C, C], f32, kind="ExternalInput")
    bq = nc.dram_tensor("bq", [C], f32, kind="ExternalInput")
    wk = nc.dram_tensor("wk", [C, C], f32, kind="ExternalInput")
    bk = nc.dram_tensor("bk", [C], f32, kind="ExternalInput")
    wv = nc.dram_tensor("wv", [C, C], f32, kind="ExternalInput")
    bv = nc.dram_tensor("bv", [C], f32, kind="ExternalInput")
    gamma = nc.dram_tensor("gamma", [1], f32, kind="ExternalInput")
    k_w1 = nc.dram_tensor("k_w1", [C, C], f32, kind="ExternalInput")
    k_w2 = nc.dram_tensor("k_w2", [C * 9, C], f32, kind="ExternalInput")
    conv_w = nc.dram_tensor("conv_w", [C, C], f32, kind="ExternalInput")
    conv_b = nc.dram_tensor("conv_b", [C], f32, kind="ExternalInput")
    ca_w1 = nc.dram_tensor("ca_w1", [C // 8, C], f32, kind="ExternalInput")
    ca_w2 = nc.dram_tensor("ca_w2", [C, C // 8], f32, kind="ExternalInput")
    out = nc.dram_tensor("out", [BPC, C, h, w], f32, kind="ExternalOutput")

    featv = feat[:, :, :, :].rearrange("b c h w -> (b c) (h w)")
    outv = out[:, :, :, :].rearrange("b c h w -> (b c) (h w)")

    with tile.TileContext(nc) as tc, contextlib.ExitStack() as ctx:
        sing = ctx.enter_context(tc.tile_pool(name="sing", bufs=1))
        work = ctx.enter_context(tc.tile_pool(name="work", bufs=3))
        dr = ctx.enter_context(tc.tile_pool(name="dr", bufs=1, space="DRAM"))
        ps_v = ctx.enter_context(tc.tile_pool(name="ps_v", bufs=1, space="PSUM"))
        ps_d = ctx.enter_context(tc.tile_pool(name="ps_d", bufs=3, space="PSUM"))
        ps_o = ctx.enter_context(tc.tile_pool(name="ps_o", bufs=2, space="PSUM"))

        def emit():
            # ------------- constants / weight prep (no feat dependency) -----
            ident_b = sing.tile([128, 128], bf16)
            make_identity(nc, ident_b[:, :])

            def load_T(src_dram, rows, cols, name):
                """Transposed load: DRAM [rows, cols] -> SBUF [cols, rows]
                via a strided AP (tiny tensors; cost irrelevant)."""
                t = sing.tile([cols, rows], f32, tag=f"T{name}")
                ap = bass.AP(tensor=src_dram[:, :].tensor, offset=0,
                             ap=[[1, cols], [cols, rows]])
                nc.sync.dma_start(out=t[:, :], in_=ap)
                return t

            def blkdiag(tsb, rows, cols, dtype=f32, name=""):
                """[128,128] block-diagonal from tsb ([rows, cols]): one block
                per batch at (b*64, b*64)."""
                blk = sing.tile([128, 128], dtype, tag=f"blk{name}")
                nc.gpsimd.memset(blk[:, :], 0.0)
                nc.vector.tensor_copy(blk[0:rows, 0:cols], tsb[:, :])
                nc.sync.dma_start(out=blk[64:64 + rows, 64:64 + cols],
                                  in_=tsb[:, :])
                return blk

            wqT = load_T(wq, 64, 64, "wq")
            wkT = load_T(wk, 64, 64, "wk")
            wvT = load_T(wv, 64, 64, "wv")
            k_w1T = load_T(k_w1, 64, 64, "kw1")
            conv_wT = load_T(conv_w, 64, 64, "cw")
            ca_w1T = load_T(ca_w1, 8, 64, "ca1")      # [64, 8]
            ca_w2T = load_T(ca_w2, 64, 8, "ca2")      # [8, 64]

            BQ = blkdiag(wqT, 64, 64, name="q")
            BK = blkdiag(wkT, 64, 64, name="k")
            BV = blkdiag(wvT, 64, 64, name="v")
            BW1 = blkdiag(k_w1T, 64, 64, name="w1")
            BA1 = blkdiag(ca_w1T, 64, 8, name="a1")
            BA2 = blkdiag(ca_w2T, 8, 64, name="a2")
            conv_wTb = sing.tile([64, 64], bf16)
            nc.vector.tensor_copy(conv_wTb[:, :], conv_wT[:, :])
            CB = blkdiag(conv_wTb, 64, 64, dtype=bf16, name="cw")

            # per-tap kernel-predictor weights: BK2[t][(b,j),(b,c)] =
            # k_w2[c*9+t, j] so that k_tap[:, t] = BK2[t].T @ hid
            BK2 = []
            for t in range(9):
                w2t = sing.tile([64, 64], f32, tag=f"w2T{t}")
                ap = bass.AP(tensor=k_w2[:, :].tensor, offset=t * 64,
                             ap=[[1, 64], [9 * 64, 64]])
                nc.sync.dma_start(out=w2t[:, :], in_=ap)
                BK2.append(blkdiag(w2t, 64, 64, name=f"k2_{t}"))

            # per-partition vectors
            def bcast_c(src, name):
                t = sing.tile([128, 1], f32, tag=f"pc{name}")
                ap = bass.AP(tensor=src[:].tensor, offset=0,
                             ap=[[0, BPC], [1, C]])
                nc.gpsimd.dma_start(out=t[:, 0:1], in_=ap)
                return t

            bq_pc = bcast_c(bq, "bq")
            bk_pc = bcast_c(bk, "bk")
            bv_pc = bcast_c(bv, "bv")
            conv_b_pc = bcast_c(conv_b, "cb")
            gamma_pc = sing.tile([128, 1], f32)
            nc.gpsimd.dma_start(
                out=gamma_pc[:, 0:1],
                in_=bass.AP(tensor=gamma[:].tensor, offset=0,
                            ap=[[0, 128], [1, 1]]))
            deg_pc = sing.tile([128, 1], f32)
            nc.sync.dma_start(out=deg_pc[:, 0:1],
                              in_=deg[:, :].rearrange("b c -> (b c)")
                              .rearrange("(p one) -> p one", one=1))

            def vec_mm(blk_w, rhs_pc, name):
                """[128,1] = blkdiag.T @ rhs (tiny matmul), result in PSUM."""
                p = ps_v.tile([128, 1], f32, tag="vec")
                nc.tensor.matmul(p[:, 0:1], blk_w[:, :], rhs_pc[:, 0:1],
                                 start=True, stop=True)
                return p

            def lrelu_vec(psum_in, name):
                """lrelu on a [128,1] psum -> sbuf f32: 0.9*relu(x) + 0.1*x."""
                tr = work.tile([128, 1], f32, tag=f"lr{name}")
                nc.scalar.activation(out=tr[:, 0:1], in_=psum_in[:, 0:1],
                                     func=AF.Relu, bias=0.0, scale=0.9)
                o = sing.tile([128, 1], f32, tag=f"lro{name}")
                nc.vector.scalar_tensor_tensor(o[:, 0:1], psum_in[:, 0:1], 0.1,
                                               tr[:, 0:1], op0=OP.mult,
                                               op1=OP.add)
                return o

            # --------- channel attention (depends only on deg) ---------------
            a0 = vec_mm(BA1, deg_pc, "a0")
            t_pr = lrelu_vec(a0, "a0")
            a1 = vec_mm(BA2, t_pr, "a1")
            att_pc = sing.tile([128, 1], f32)
            nc.scalar.activation(out=att_pc[:, 0:1], in_=a1[:, 0:1],
                                 func=AF.Sigmoid, bias=0.0, scale=1.0)

            # --------- kk / v (depend only on deg) ---------------------------
            kk0 = vec_mm(BK, deg_pc, "kk")
            kk_pi = sing.tile([128, 1], f32)
            nc.scalar.activation(out=kk_pi[:, 0:1], in_=kk0[:, 0:1],
                                 func=AF.Identity, bias=bk_pc[:, 0:1],
                                 scale=1.0)
            v0 = vec_mm(BV, deg_pc, "v")
            v_pi = sing.tile([128, 1], f32)
            nc.scalar.activation(out=v_pi[:, 0:1], in_=v0[:, 0:1],
                                 func=AF.Identity, bias=bv_pc[:, 0:1],
                                 scale=1.0)

            def rep64(src_pc, name):
                """[128,1] (p=(b,i)) -> [128,64] tile whose row (b,j) is
                src[b*64 : b*64+64] (replicated across j)."""
                d = dr.tile([128], f32, tag=f"dr{name}")
                nc.sync.dma_start(out=d[:], in_=src_pc[:, 0:1])
                rep = sing.tile([128, 64], f32, tag=f"rep{name}")
                ap = bass.AP(tensor=d[:].tensor, offset=d[:].offset,
                             ap=[[64, BPC], [0, C], [1, C]])
                nc.sync.dma_start(out=rep[:, :], in_=ap)
                return rep

            kk_rep = rep64(kk_pi, "kk")
            v_rep = rep64(v_pi, "v")

            # ------------- pass 1: load feat -> bf16 cache + row sums --------
            cache = sing.tile([128, cn], bf16)
            # zero pads: top row, col 0 of every row, bottom row + tail
            nc.gpsimd.memset(cache[:, 0:pw], 0.0)
            nc.gpsimd.memset(
                cache[:, pw:ph * pw].rearrange("p (h w) -> p h w",
                                               w=pw)[:, :, 0:1], 0.0)
            nc.gpsimd.memset(cache[:, (ph - 1) * pw:cn], 0.0)

            partials = sing.tile([128, n_slabs], f32)
            for i in range(n_slabs):
                base = (i * lr + 1) * pw + 1
                dst = cache[:, base - 1:base - 1 + lr * pw] \
                    .rearrange("p (h w) -> p h w", w=pw)[:, :, 1:1 + w]
                src = featv[:, i * lr * w:(i + 1) * lr * w] \
                    .rearrange("p (h w) -> p h w", w=w)
                nc.gpsimd.dma_start(out=dst, in_=src)  # casting DMA f32->bf16
                nc.vector.reduce_sum(partials[:, i:i + 1], dst,
                                     axis=mybir.AxisListType.XY)

            feat_ave = sing.tile([128, 1], f32)
            nc.vector.reduce_sum(feat_ave[:, 0:1], partials[:, :],
                                 axis=mybir.AxisListType.X)
            nc.vector.tensor_scalar_mul(feat_ave[:, 0:1], feat_ave[:, 0:1],
                                        1.0 / npx)

            # ------------- CMA + kernel-predictor MLP ------------------------
            q0 = vec_mm(BQ, feat_ave, "q")
            q_pj = sing.tile([128, 1], f32)
            nc.scalar.activation(out=q_pj[:, 0:1], in_=q0[:, 0:1],
                                 func=AF.Identity, bias=bq_pc[:, 0:1],
                                 scale=1.0)

            # softmax without max-subtraction: |energy| is O(1) by construction
            energy = sing.tile([128, C], f32)
            nc.vector.tensor_scalar_mul(energy[:, :], kk_rep[:, :],
                                        q_pj[:, 0:1])
            ee = sing.tile([128, C], f32)
            nc.scalar.activation(out=ee[:, :], in_=energy[:, :], func=AF.Exp,
                                 bias=0.0, scale=1.0)
            es = sing.tile([128, 1], f32)
            nc.vector.reduce_sum(es[:, 0:1], ee[:, :],
                                 axis=mybir.AxisListType.X)
            erc = sing.tile([128, 1], f32)
            nc.vector.reciprocal(erc[:, 0:1], es[:, 0:1])
            attn = sing.tile([128, C], f32)
            nc.vector.tensor_scalar_mul(attn[:, :], ee[:, :], erc[:, 0:1])
            prod = sing.tile([128, C], f32)
            nc.vector.tensor_mul(prod[:, :], attn[:, :], v_rep[:, :])
            cma = sing.tile([128, 1], f32)
            nc.vector.reduce_sum(cma[:, 0:1], prod[:, :],
                                 axis=mybir.AxisListType.X)
            emb = sing.tile([128, 1], f32)
            nc.vector.scalar_tensor_tensor(emb[:, 0:1], cma[:, 0:1],
                                           gamma_pc[:, 0:1], deg_pc[:, 0:1],
                                           op0=OP.mult, op1=OP.add)

            hid0 = vec_mm(BW1, emb, "hid")
            hid_pc = lrelu_vec(hid0, "hid")

            # k_tap[:, t] = BK2[t].T @ hid  -> all 9 taps into one PSUM bank
            ktp = ps_v.tile([128, 9], f32, tag="ktap")
            for t in range(9):
                nc.tensor.matmul(ktp[:, t:t + 1], BK2[t][:, :],
                                 hid_pc[:, 0:1], start=True, stop=True)
            k_tap = sing.tile([128, 9], f32)
            nc.scalar.copy(k_tap[:, :], ktp[:, :])

            diags = []
            for t in range(9):
                dg = sing.tile([128, 128], bf16, tag=f"diag{t}")
                nc.vector.tensor_scalar_mul(dg[:, :], ident_b[:, :],
                                            k_tap[:, t:t + 1])
                diags.append(dg)

            # ------------- main loop: 2 output rows per group ----------------
            cap = cache[:, :]

            def slab(r0, off):
                # [128, 2, w] strided view at flat offset r0*pw + 1 + off
                base = r0 * pw + 1 + off
                return bass.AP(tensor=cap.tensor, offset=cap.offset + base,
                               ap=[list(cap.ap[0]), [pw, 2], [1, w]])

            for g in range(n_groups):
                r0 = 2 * g + 1
                pd = ps_d.tile([128, 2 * w], f32, tag="pd")
                for idx in range(9):
                    ky, kx = idx // 3, idx % 3
                    off = (ky - 1) * pw + (kx - 1)
                    nc.tensor.matmul(pd[:, :], diags[idx][:, :], slab(r0, off),
                                     start=(idx == 0), stop=(idx == 8))
                t_relu = work.tile([128, 2 * w], f32, tag="t_relu")
                nc.scalar.activation(out=t_relu[:, :], in_=pd[:, :],
                                     func=AF.Relu, bias=0.0, scale=0.9)
                y = work.tile([128, 2 * w], bf16, tag="y")
                nc.vector.scalar_tensor_tensor(y[:, :], pd[:, :], 0.1,
                                               t_relu[:, :], op0=OP.mult,
                                               op1=OP.add)
                po = ps_o.tile([128, 2 * w], f32, tag="po")
                nc.tensor.matmul(po[:, :], CB[:, :], y[:, :],
                                 start=True, stop=True)
                t1 = work.tile([128, 2 * w], f32, tag="t1")
                nc.scalar.activation(out=t1[:, :], in_=slab(r0, 0),
                                     func=AF.Identity, bias=conv_b_pc[:, 0:1],
                                     scale=att_pc[:, 0:1])
                out_s = work.tile([128, 2 * w], f32, tag="out_s")
                nc.vector.tensor_add(out_s[:, :], t1[:, :], po[:, :])
                nc.sync.dma_start(out=outv[:, g * 2 * w:(g + 1) * 2 * w],
                                  in_=out_s[:, :])

        if loop_reps > 1:
            with tc.For_i(0, loop_reps, 1):
                emit()
        else:
            emit()

    nc.finalize()
    return nc


_NC_CACHE = {}


def _get_nc(h, w):
    if (h, w) not in _NC_CACHE:
        _NC_CACHE[(h, w)] = build_nc(h, w)
    return _NC_CACHE[(h, w)]


def kernel(**inputs):
    from concourse.bass_utils import run_bass_kernel_spmd

    feat = np.ascontiguousarray(inputs["feat"], dtype=np.float32)
    deg = np.ascontiguousarray(inputs["deg"], dtype=np.float32)
    b, c, h, w = feat.shape
    nc = _get_nc(h, w)

    shared = {k: np.ascontiguousarray(np.asarray(v), dtype=np.float32)
              for k, v in inputs.items() if k not in ("feat", "deg")}
    in_maps = []
    for k in range(NCORES):
        m = dict(shared)
        m["feat"] = feat[k * BPC:(k + 1) * BPC]
        m["deg"] = deg[k * BPC:(k + 1) * BPC]
        in_maps.append(m)

    res = run_bass_kernel_spmd(nc, in_maps, core_ids=list(range(NCORES)))
    return np.concatenate([r["out"] for r in res.results], axis=0)
